# revision 10
# baseline (speedup 1.0000x reference)
"""Trainium2 Bass kernel for nn_ContrastLoss (supervised-contrastive loss).

Reference computation (B=1024, D=128, C=100, K=32768, N=B+K=33792):
    l   = concat(labels, queue_label.T)          # [N, C]
    w   = labels @ l.T                           # [B, N] shared-class counts
    sim = query @ concat(keys, queue.T).T / T    # [B, N]
    logits = sim - rowmax(sim)
    denom  = sum(exp(logits) * logits_mask, 1)   # logits_mask zeros keys-diag
    loss = -(T/BT) * sqrt(w/max(w)) * (logits - log(denom))

Structure ("recompute", v3):
  * Data-parallel over B: core c owns rows [c*128, (c+1)*128), all N cols.
  * Softmax stabilizer = 1.0 (inputs are L2-normalized), kills rowmax.
  * Self-diagonal handled via host-computed qk_i = q_i . k_i: subtract
    exp((qk-1)/T) from the denominator (no masked pass).
  * Phase A (chunks of 2048): sim matmul (bf16) -> PSUM; ACT Exp reads
    PSUM directly (sole reader) -> bf16 e_scr; the idle DVE row-sums
    e_scr into acc (beats ACT accum_out reads by 3us).  Raw sims are
    NOT evacuated -- phase B re-runs the matmul from the SBUF-resident
    rsim, which deletes the whole DVE cast pass (the old bottleneck).
  * Phase B (chunks of 1024, two double-buffered PSUM pools so the
    matmuls stay OFF the ACT/DVE critical path): w matmul (fp8, exact
    for 0/1 labels) -> Sqrt -> sT; sim matmul again -> psum; one DVE
    scalar_tensor_tensor computes o = (raw - tc) * sT straight from
    PSUM (o = -loss; host negates).
  * Sqrt's scale comes from an AP derived from ln(denom) purely to pin
    the ACT queue order Ln -> Sqrt (avoids ACT-table thrash), and
    output DMAs issue from the idle Pool sequencer so they never queue
    behind input DMAs on SP.
"""

import numpy as np
import ml_dtypes

import concourse.bass as bass
import concourse.mybir as mybir
import concourse.tile as tile
from concourse import bacc
from concourse.bass_utils import run_bass_kernel_spmd

F32 = mybir.dt.float32
BF16 = mybir.dt.bfloat16
FP8 = mybir.dt.float8e4
ALU = mybir.AluOpType
ACTF = mybir.ActivationFunctionType

B, D, C, KQ = 1024, 128, 100, 32768
N = B + KQ                  # 33792 similarity columns
NCORES = 8
ROWS = B // NCORES          # 128 rows per core
STAB = 1.0                  # softmax stabilizer m (raw sim values in [-1, 1])

CHA = 2048                  # phase A steady-state chunk: 4 PSUM banks
# Graduated ramp-in (512/512/1024) so the first Exp starts ~4us earlier,
# then 2048-chunks; N = 33792 = 512+512+1024 + 15*2048 + 1024.
_a_sizes = [512, 512, 1024] + [2048] * 15 + [1024]
assert sum(_a_sizes) == N
ACHUNKS = []
_off = 0
for _s in _a_sizes:
    ACHUNKS.append((_off, _s))
    _off += _s
CHB = 1024                  # phase B chunk: 2 PSUM banks
BCHUNKS = [(i * CHB, CHB) for i in range(N // CHB)]


def _build_nc(Tf: float, BTf: float, wmax: float):
    nc = bacc.Bacc("TRN2", target_bir_lowering=False, debug=False,
                   num_devices=NCORES)

    qTb_d = nc.dram_tensor("qTb", [D, ROWS], BF16, kind="ExternalInput")
    labTb_d = nc.dram_tensor("labTb", [C, ROWS], FP8, kind="ExternalInput")
    qk_d = nc.dram_tensor("qk", [ROWS, 1], F32, kind="ExternalInput")
    rsim_d = nc.dram_tensor("rsim", [D, N], BF16, kind="ExternalInput")
    rw_d = nc.dram_tensor("rw", [C, N], FP8, kind="ExternalInput")
    out_d = nc.dram_tensor("out", [ROWS, N], BF16, kind="ExternalOutput")

    sq_scale = 1.0 / (BTf * BTf * max(wmax, 1.0))

    with tile.TileContext(nc) as tc:
        with (
            tc.tile_pool(name="const", bufs=1) as const,
            tc.tile_pool(name="escr", bufs=2) as escr_p,
            tc.tile_pool(name="sT", bufs=2) as sT_p,
            tc.tile_pool(name="outp", bufs=3) as outp,
        ):
          with (
            tc.tile_pool(name="psA", bufs=2, space="PSUM") as psA,
          ):
            # ---- resident inputs.  qTb + rsim chunk 0 land first so the
            # first matmul starts early; the rsim tail uses 4 big DMAs to
            # save SP sequencer issue time.  rw issues from the Pool
            # sequencer and is only needed once phase B starts. ------------
            qTb = const.tile([D, ROWS], BF16)
            nc.sync.dma_start(out=qTb[:], in_=qTb_d[:])
            rsim = const.tile([D, N], BF16)
            for base, n in ACHUNKS:
                nc.sync.dma_start(out=rsim[:, base:base + n],
                                  in_=rsim_d[:, base:base + n])
            labTb = const.tile([C, ROWS], FP8)
            nc.sync.dma_start(out=labTb[:], in_=labTb_d[:])
            qk = const.tile([ROWS, 1], F32)
            nc.sync.dma_start(out=qk[:], in_=qk_d[:])
            # rw rides the (otherwise idle) DVE sequencer's DMA queue in 4
            # big transfers so it is fully resident before phase B without
            # queueing behind the JIT rsim chunks on SP.
            rw = const.tile([C, N], FP8)
            for i in range(4):
                a = i * (N // 4)
                nc.scalar.dma_start(out=rw[:, a:a + N // 4],
                                    in_=rw_d[:, a:a + N // 4])

            ebias = const.tile([ROWS, 1], F32)
            nc.vector.memset(ebias, -STAB / Tf)
            zbias = const.tile([ROWS, 1], F32)
            nc.vector.memset(zbias, 0.0)

            # ---- phase A: sim matmul -> Exp(PSUM) with rowsum accum ------
            acc = const.tile([ROWS, len(ACHUNKS)], F32)
            for k, (base, n) in enumerate(ACHUNKS):
                ps = psA.tile([ROWS, n], F32, tag="pa")
                for o in range(0, n, 512):
                    nc.tensor.matmul(ps[:, o:o + 512], qTb[:],
                                     rsim[:, base + o:base + o + 512],
                                     start=True, stop=True)
                e_scr = escr_p.tile([ROWS, n], BF16, tag="e")
                nc.scalar.activation(e_scr[:], ps[:], ACTF.Exp,
                                     bias=ebias[:], scale=1.0 / Tf,
                                     accum_out=acc[:, k:k + 1])

            # ---- self-diagonal term + per-row constant tc ----------------
            eself = const.tile([ROWS, 1], F32)
            nc.scalar.activation(eself[:], qk[:], ACTF.Exp,
                                 bias=ebias[:], scale=1.0 / Tf)
            dnsum = const.tile([ROWS, 1], F32)
            nc.vector.tensor_reduce(dnsum[:], acc[:], axis=mybir.AxisListType.X,
                                    op=ALU.add)
            denom = const.tile([ROWS, 1], F32)
            nc.vector.tensor_sub(denom[:], dnsum[:], eself[:])
            lnd = const.tile([ROWS, 1], F32)
            nc.scalar.activation(lnd[:], denom[:], ACTF.Ln, bias=zbias[:])
            tc_row = const.tile([ROWS, 1], F32)
            nc.vector.tensor_scalar(tc_row[:], lnd[:], Tf, STAB,
                                    op0=ALU.mult, op1=ALU.add)
            # Sqrt takes its scale from an AP derived from lnd purely to pin
            # the ACT queue order Ln -> Sqrt(0..): otherwise the scheduler
            # runs an early Sqrt before Ln and thrashes the ACT table set.
            sq_ap = const.tile([ROWS, 1], F32)
            nc.vector.tensor_scalar(sq_ap[:], lnd[:], 0.0, sq_scale,
                                    op0=ALU.mult, op1=ALU.add)

          with (
            tc.tile_pool(name="psW", bufs=2, space="PSUM") as psW,
            tc.tile_pool(name="psB", bufs=2, space="PSUM") as psB,
          ):
            # ---- phase B: w matmul -> sT; sim re-matmul -> fused output --
            for k, (base, n) in enumerate(BCHUNKS):
                psw = psW.tile([ROWS, n], F32, tag="pw")
                for o in range(0, n, 512):
                    nc.tensor.matmul(psw[:, o:o + 512], labTb[:],
                                     rw[:, base + o:base + o + 512],
                                     start=True, stop=True)
                sT = sT_p.tile([ROWS, n], BF16, tag="s")
                nc.scalar.activation(sT[:], psw[:], ACTF.Sqrt,
                                     bias=zbias[:], scale=sq_ap[:])
                psb = psB.tile([ROWS, n], F32, tag="pb")
                for o in range(0, n, 512):
                    nc.tensor.matmul(psb[:, o:o + 512], qTb[:],
                                     rsim[:, base + o:base + o + 512],
                                     start=True, stop=True)
                o_t = outp.tile([ROWS, n], BF16, tag="o")
                # o = (raw - tc) * sT = -loss; negated on the host.
                nc.vector.scalar_tensor_tensor(
                    o_t[:], psb[:], tc_row[:], sT[:],
                    op0=ALU.subtract, op1=ALU.mult,
                )
                # Output DMAs issue from the idle Pool sequencer so they don't
                # queue behind the input DMAs on SP.
                nc.gpsimd.dma_start(out=out_d[:, base:base + n], in_=o_t[:])
    nc.compile()
    return nc


def _host_prep(query, keys, labels, queue, queue_label):
    bf16 = ml_dtypes.bfloat16
    fp8 = ml_dtypes.float8_e4m3
    query = np.asarray(query, np.float32)
    keys = np.asarray(keys, np.float32)
    labels = np.asarray(labels, np.float32)
    queue = np.asarray(queue, np.float32)
    queue_label = np.asarray(queue_label, np.float32)

    qT = query.T                                        # [D, B]
    labT = labels.T                                     # [C, B]
    qk = (query * keys).sum(axis=1, keepdims=True).astype(np.float32)
    rsim = np.ascontiguousarray(
        np.concatenate([keys.T, queue], axis=1)).astype(bf16)   # [D, N]
    rw = np.ascontiguousarray(
        np.concatenate([labT, queue_label], axis=1)).astype(fp8)  # [C, N]

    in_maps = []
    for c in range(NCORES):
        blk = slice(c * ROWS, (c + 1) * ROWS)
        in_maps.append({
            "qTb": np.ascontiguousarray(qT[:, blk]).astype(bf16),
            "labTb": np.ascontiguousarray(labT[:, blk]).astype(fp8),
            "qk": np.ascontiguousarray(qk[blk]),
            "rsim": rsim,
            "rw": rw,
        })
    return in_maps


def _gather_output(results):
    out = np.empty((B, N), np.float32)
    for c in range(NCORES):
        out[c * ROWS:(c + 1) * ROWS, :] = -results[c]["out"].astype(np.float32)
    return out


def kernel(query, keys, labels, queue, queue_label, K, T, BT, **_unused):
    Tf = float(np.asarray(T))
    BTf = float(np.asarray(BT))
    labels = np.asarray(labels, np.float32)
    wmax = float(labels.sum(axis=1).max())
    nc = _build_nc(Tf, BTf, wmax)
    in_maps = _host_prep(query, keys, labels, queue, queue_label)
    res = run_bass_kernel_spmd(nc, in_maps, list(range(NCORES)))
    return _gather_output(res.results)


# Re-usable entry for test.py: returns (output, BassKernelResults) so the
# harness there can pull exec_time_ns / profile out of a traced run.
def kernel_traced(query, keys, labels, queue, queue_label, K, T, BT,
                  trace=False, **run_kwargs):
    Tf = float(np.asarray(T))
    BTf = float(np.asarray(BT))
    labels = np.asarray(labels, np.float32)
    wmax = float(labels.sum(axis=1).max())
    nc = _build_nc(Tf, BTf, wmax)
    in_maps = _host_prep(query, keys, labels, queue, queue_label)
    res = run_bass_kernel_spmd(nc, in_maps, list(range(NCORES)),
                               trace=trace, **run_kwargs)
    return _gather_output(res.results), res


# revision 11
# speedup vs baseline: 1.0445x; 1.0445x over previous
"""Trainium2 Bass kernel for nn_ContrastLoss (supervised-contrastive loss).

Reference computation (B=1024, D=128, C=100, K=32768, N=B+K=33792):
    l   = concat(labels, queue_label.T)          # [N, C]
    w   = labels @ l.T                           # [B, N] shared-class counts
    sim = query @ concat(keys, queue.T).T / T    # [B, N]
    logits = sim - rowmax(sim)
    denom  = sum(exp(logits) * logits_mask, 1)   # logits_mask zeros keys-diag
    loss = -(T/BT) * sqrt(w/max(w)) * (logits - log(denom))

Structure ("recompute", v3):
  * Data-parallel over B: core c owns rows [c*128, (c+1)*128), all N cols.
  * Softmax stabilizer = 1.0 (inputs are L2-normalized), kills rowmax.
  * Self-diagonal handled via host-computed qk_i = q_i . k_i: subtract
    exp((qk-1)/T) from the denominator (no masked pass).
  * Phase A (chunks of 2048): sim matmul (bf16) -> PSUM; ACT Exp reads
    PSUM directly (sole reader) -> bf16 e_scr; the idle DVE row-sums
    e_scr into acc (beats ACT accum_out reads by 3us).  Raw sims are
    NOT evacuated -- phase B re-runs the matmul from the SBUF-resident
    rsim, which deletes the whole DVE cast pass (the old bottleneck).
  * Phase B (chunks of 1024, two double-buffered PSUM pools so the
    matmuls stay OFF the ACT/DVE critical path): w matmul (fp8, exact
    for 0/1 labels) -> Sqrt -> sT; sim matmul again -> psum; one DVE
    scalar_tensor_tensor computes o = (raw - tc) * sT straight from
    PSUM (o = -loss; host negates).
  * Sqrt's scale comes from an AP derived from ln(denom) purely to pin
    the ACT queue order Ln -> Sqrt (avoids ACT-table thrash), and
    output DMAs issue from the idle Pool sequencer so they never queue
    behind input DMAs on SP.
"""

import numpy as np
import ml_dtypes

import concourse.bass as bass
import concourse.mybir as mybir
import concourse.tile as tile
from concourse import bacc
from concourse.bass_utils import run_bass_kernel_spmd

F32 = mybir.dt.float32
BF16 = mybir.dt.bfloat16
FP8 = mybir.dt.float8e4
ALU = mybir.AluOpType
ACTF = mybir.ActivationFunctionType

B, D, C, KQ = 1024, 128, 100, 32768
N = B + KQ                  # 33792 similarity columns
NCORES = 8
ROWS = B // NCORES          # 128 rows per core
STAB = 1.0                  # softmax stabilizer m (raw sim values in [-1, 1])

CHA = 2048                  # phase A steady-state chunk: 4 PSUM banks
# Graduated ramp-in (512/512/1024) so the first Exp starts ~4us earlier,
# then 2048-chunks; N = 33792 = 512+512+1024 + 15*2048 + 1024.
_a_sizes = [512, 512, 1024] + [2048] * 15 + [1024]
assert sum(_a_sizes) == N
ACHUNKS = []
_off = 0
for _s in _a_sizes:
    ACHUNKS.append((_off, _s))
    _off += _s
CHB = 1024                  # phase B chunk: 2 PSUM banks
BCHUNKS = [(i * CHB, CHB) for i in range(N // CHB)]


def _build_nc(Tf: float, BTf: float, wmax: float):
    nc = bacc.Bacc("TRN2", target_bir_lowering=False, debug=False,
                   num_devices=NCORES)

    qTb_d = nc.dram_tensor("qTb", [D, ROWS], BF16, kind="ExternalInput")
    labTb_d = nc.dram_tensor("labTb", [C, ROWS], FP8, kind="ExternalInput")
    qk_d = nc.dram_tensor("qk", [ROWS, 1], F32, kind="ExternalInput")
    rsim_d = nc.dram_tensor("rsim", [D, N], BF16, kind="ExternalInput")
    rw_d = nc.dram_tensor("rw", [C, N], FP8, kind="ExternalInput")
    out_d = nc.dram_tensor("out", [ROWS, N], BF16, kind="ExternalOutput")

    sq_scale = 1.0 / (BTf * BTf * max(wmax, 1.0))

    with tile.TileContext(nc) as tc:
        with (
            tc.tile_pool(name="const", bufs=1) as const,
            tc.tile_pool(name="escr", bufs=2) as escr_p,
            tc.tile_pool(name="sT", bufs=2) as sT_p,
            tc.tile_pool(name="outp", bufs=3) as outp,
        ):
          with (
            tc.tile_pool(name="psA", bufs=2, space="PSUM") as psA,
          ):
            # ---- resident inputs.  qTb + rsim chunk 0 land first so the
            # first matmul starts early; the rsim tail uses 4 big DMAs to
            # save SP sequencer issue time.  rw issues from the Pool
            # sequencer and is only needed once phase B starts. ------------
            qTb = const.tile([D, ROWS], BF16)
            nc.sync.dma_start(out=qTb[:], in_=qTb_d[:])
            labTb = const.tile([C, ROWS], FP8)
            nc.sync.dma_start(out=labTb[:], in_=labTb_d[:])
            qk = const.tile([ROWS, 1], F32)
            nc.sync.dma_start(out=qk[:], in_=qk_d[:])
            # rsim chunks stream JIT for phase A; rw chunks ride the same SP
            # queue interleaved two chunks behind, soaking up the leftover
            # DMA bandwidth so rw is fully resident before phase B starts
            # (big up-front rw transfers head-of-line block rsim chunk 0).
            rsim = const.tile([D, N], BF16)
            rw = const.tile([C, N], FP8)
            RWCH = [(i * 2048, 2048) for i in range(N // 2048)] + [
                (N - N % 2048, N % 2048)] if N % 2048 else [
                (i * 2048, 2048) for i in range(N // 2048)]
            rw_iter = iter(RWCH)
            for k, (base, n) in enumerate(ACHUNKS):
                nc.sync.dma_start(out=rsim[:, base:base + n],
                                  in_=rsim_d[:, base:base + n])
                if k >= 2:
                    try:
                        rb, rn = next(rw_iter)
                        nc.sync.dma_start(out=rw[:, rb:rb + rn],
                                          in_=rw_d[:, rb:rb + rn])
                    except StopIteration:
                        pass
            for rb, rn in rw_iter:
                nc.sync.dma_start(out=rw[:, rb:rb + rn],
                                  in_=rw_d[:, rb:rb + rn])

            ebias = const.tile([ROWS, 1], F32)
            nc.vector.memset(ebias, -STAB / Tf)
            zbias = const.tile([ROWS, 1], F32)
            nc.vector.memset(zbias, 0.0)

            # ---- phase A: sim matmul -> Exp(PSUM) with rowsum accum ------
            acc = const.tile([ROWS, len(ACHUNKS)], F32)
            for k, (base, n) in enumerate(ACHUNKS):
                ps = psA.tile([ROWS, n], F32, tag="pa")
                for o in range(0, n, 512):
                    nc.tensor.matmul(ps[:, o:o + 512], qTb[:],
                                     rsim[:, base + o:base + o + 512],
                                     start=True, stop=True)
                e_scr = escr_p.tile([ROWS, n], BF16, tag="e")
                nc.scalar.activation(e_scr[:], ps[:], ACTF.Exp,
                                     bias=ebias[:], scale=1.0 / Tf,
                                     accum_out=acc[:, k:k + 1])

            # ---- self-diagonal term + per-row constant tc ----------------
            eself = const.tile([ROWS, 1], F32)
            nc.scalar.activation(eself[:], qk[:], ACTF.Exp,
                                 bias=ebias[:], scale=1.0 / Tf)
            dnsum = const.tile([ROWS, 1], F32)
            nc.vector.tensor_reduce(dnsum[:], acc[:], axis=mybir.AxisListType.X,
                                    op=ALU.add)
            denom = const.tile([ROWS, 1], F32)
            nc.vector.tensor_sub(denom[:], dnsum[:], eself[:])
            lnd = const.tile([ROWS, 1], F32)
            nc.scalar.activation(lnd[:], denom[:], ACTF.Ln, bias=zbias[:])
            tc_row = const.tile([ROWS, 1], F32)
            nc.vector.tensor_scalar(tc_row[:], lnd[:], Tf, STAB,
                                    op0=ALU.mult, op1=ALU.add)
            # Sqrt takes its scale from an AP derived from lnd purely to pin
            # the ACT queue order Ln -> Sqrt(0..): otherwise the scheduler
            # runs an early Sqrt before Ln and thrashes the ACT table set.
            sq_ap = const.tile([ROWS, 1], F32)
            nc.vector.tensor_scalar(sq_ap[:], lnd[:], 0.0, sq_scale,
                                    op0=ALU.mult, op1=ALU.add)

          with (
            tc.tile_pool(name="psW", bufs=2, space="PSUM") as psW,
            tc.tile_pool(name="psB", bufs=2, space="PSUM") as psB,
          ):
            # ---- phase B: w matmul -> sT; sim re-matmul -> fused output --
            for k, (base, n) in enumerate(BCHUNKS):
                psw = psW.tile([ROWS, n], F32, tag="pw")
                for o in range(0, n, 512):
                    nc.tensor.matmul(psw[:, o:o + 512], labTb[:],
                                     rw[:, base + o:base + o + 512],
                                     start=True, stop=True)
                sT = sT_p.tile([ROWS, n], BF16, tag="s")
                nc.scalar.activation(sT[:], psw[:], ACTF.Sqrt,
                                     bias=zbias[:], scale=sq_ap[:])
                psb = psB.tile([ROWS, n], F32, tag="pb")
                for o in range(0, n, 512):
                    nc.tensor.matmul(psb[:, o:o + 512], qTb[:],
                                     rsim[:, base + o:base + o + 512],
                                     start=True, stop=True)
                o_t = outp.tile([ROWS, n], BF16, tag="o")
                # o = (raw - tc) * sT = -loss; negated on the host.
                nc.vector.scalar_tensor_tensor(
                    o_t[:], psb[:], tc_row[:], sT[:],
                    op0=ALU.subtract, op1=ALU.mult,
                )
                # Output DMAs issue from the idle Pool sequencer so they don't
                # queue behind the input DMAs on SP.
                nc.gpsimd.dma_start(out=out_d[:, base:base + n], in_=o_t[:])
    nc.compile()
    return nc


def _host_prep(query, keys, labels, queue, queue_label):
    bf16 = ml_dtypes.bfloat16
    fp8 = ml_dtypes.float8_e4m3
    query = np.asarray(query, np.float32)
    keys = np.asarray(keys, np.float32)
    labels = np.asarray(labels, np.float32)
    queue = np.asarray(queue, np.float32)
    queue_label = np.asarray(queue_label, np.float32)

    qT = query.T                                        # [D, B]
    labT = labels.T                                     # [C, B]
    qk = (query * keys).sum(axis=1, keepdims=True).astype(np.float32)
    rsim = np.ascontiguousarray(
        np.concatenate([keys.T, queue], axis=1)).astype(bf16)   # [D, N]
    rw = np.ascontiguousarray(
        np.concatenate([labT, queue_label], axis=1)).astype(fp8)  # [C, N]

    in_maps = []
    for c in range(NCORES):
        blk = slice(c * ROWS, (c + 1) * ROWS)
        in_maps.append({
            "qTb": np.ascontiguousarray(qT[:, blk]).astype(bf16),
            "labTb": np.ascontiguousarray(labT[:, blk]).astype(fp8),
            "qk": np.ascontiguousarray(qk[blk]),
            "rsim": rsim,
            "rw": rw,
        })
    return in_maps


def _gather_output(results):
    out = np.empty((B, N), np.float32)
    for c in range(NCORES):
        out[c * ROWS:(c + 1) * ROWS, :] = -results[c]["out"].astype(np.float32)
    return out


def kernel(query, keys, labels, queue, queue_label, K, T, BT, **_unused):
    Tf = float(np.asarray(T))
    BTf = float(np.asarray(BT))
    labels = np.asarray(labels, np.float32)
    wmax = float(labels.sum(axis=1).max())
    nc = _build_nc(Tf, BTf, wmax)
    in_maps = _host_prep(query, keys, labels, queue, queue_label)
    res = run_bass_kernel_spmd(nc, in_maps, list(range(NCORES)))
    return _gather_output(res.results)


# Re-usable entry for test.py: returns (output, BassKernelResults) so the
# harness there can pull exec_time_ns / profile out of a traced run.
def kernel_traced(query, keys, labels, queue, queue_label, K, T, BT,
                  trace=False, **run_kwargs):
    Tf = float(np.asarray(T))
    BTf = float(np.asarray(BT))
    labels = np.asarray(labels, np.float32)
    wmax = float(labels.sum(axis=1).max())
    nc = _build_nc(Tf, BTf, wmax)
    in_maps = _host_prep(query, keys, labels, queue, queue_label)
    res = run_bass_kernel_spmd(nc, in_maps, list(range(NCORES)),
                               trace=trace, **run_kwargs)
    return _gather_output(res.results), res
